# revision 31
# baseline (speedup 1.0000x reference)
"""CTC loss (Keras ctc_batch_cost semantics) on 8 Trainium2 NeuronCores.

Strategy: data-parallel over the batch axis (64 sequences per core). The CTC
forward DP runs in the *linear* probability domain with periodic max-
renormalization (scaled forward algorithm), so each time step is 4 DVE ops on
a [64 batch-partitions, 161 extended-state] tile:

    y = m .* q[s-2]            (skip-transition mask multiply)
    x = q + q[s-1]
    u = x + y
    q' = (u [* 1/z]) .* g_t    (g_t = gathered per-state emission probs)

Emission probs g_t[b,s] = y_pred[b,t,ext[b,s]] are gathered per (b, t-chunk)
by GPSIMD indirect_copy in [t-partition, s-free] layout and transposed to the
chain's [b-partition, (t,s)-free] layout with SBUF->SBUF DMAs. Softmax
normalizers Z[b,t] = sum_c y_pred and the final log-combine are handled by
the scalar engine; the loss is

    loss[b] = sum_t ln Z[b,t] - sum_renorms ln z - ln(qT[S-1] + qT[S-2]).
"""

import functools
import os
import sys

import numpy as np

B, T, C, L = 512, 512, 128, 80
S = 2 * L + 1  # 161
BLANK = C - 1
EPS = 1e-7
NCORES = 8
BPC = B // NCORES  # 64 sequences per core
TC = 64  # time-chunk
NCHUNK = T // TC  # 8
NPAIR = BPC // 2  # 32 pair-tiles (2 sequences each) per chunk
IDXW = 12  # wrapped-index columns, padded even so slices stay 4B-aligned
RENORM = 8  # renormalize every 8 steps
SPAD = S + 2  # zero-padded state row
SG = S + 3  # gather width padded to a multiple of 4 (ISA requirement)


def _emit_kernel(ctx, tc, ypred, idxt, maskt, losst, variant="full"):
    import concourse.bass as bass  # noqa: F401
    import concourse.mybir as mybir

    nc = tc.nc
    f32 = mybir.dt.float32
    Alu = mybir.AluOpType
    Act = mybir.ActivationFunctionType

    singles = ctx.enter_context(tc.tile_pool(name="singles", bufs=1))
    ypool = ctx.enter_context(tc.tile_pool(name="ypool", bufs=2))
    gpool = ctx.enter_context(tc.tile_pool(name="gpool", bufs=2))
    g2pool = ctx.enter_context(tc.tile_pool(name="g2pool", bufs=4))
    zscr = ctx.enter_context(tc.tile_pool(name="zscr", bufs=2))
    small = ctx.enter_context(tc.tile_pool(name="small", bufs=2))
    finp = ctx.enter_context(tc.tile_pool(name="finp", bufs=8))
    psump = ctx.enter_context(tc.tile_pool(name="psum", bufs=1, space="PSUM"))

    # --- constants loaded once -------------------------------------------
    idx_sb = singles.tile([128, NPAIR * IDXW], mybir.dt.uint16)
    nc.sync.dma_start(out=idx_sb[:, :], in_=idxt)
    m_sb = singles.tile([BPC, S], f32)
    nc.sync.dma_start(out=m_sb[:, :], in_=maskt)
    # pre-touch idx on GPSIMD so no gather has to wait for its load DMA
    idx_scr = singles.tile([16, 1], mybir.dt.uint16)
    nc.gpsimd.tensor_copy(out=idx_scr[:, :], in_=idx_sb[0:16, 0:1])

    # Z accumulator: col = chunk*NPAIR + pair, value = sum_c y_pred for the
    # 64 t's x 2 b's living in that pair-tile's partitions.
    zbig = singles.tile([128, NCHUNK * NPAIR], f32)
    # half-selector for the final partition-axis reduction via PE
    halfsel = singles.tile([128, 2], f32)
    nc.vector.memset(halfsel[:, :], 0.0)
    nc.vector.memset(halfsel[0:64, 0:1], 1.0)
    nc.vector.memset(halfsel[64:128, 1:2], 1.0)

    # --- producers: load y chunks, Z row-sums, gathers, b<->t swap -------
    gtiles = []
    for ch in range(NCHUNK):
        t0 = ch * TC
        ytile = ypool.tile([128, NPAIR, C], f32, tag="ychunk")
        for j in range(NPAIR):
            nc.sync.dma_start(out=ytile[0:64, j, :], in_=ypred[2 * j, t0 : t0 + TC, :])
            nc.sync.dma_start(
                out=ytile[64:128, j, :], in_=ypred[2 * j + 1, t0 : t0 + TC, :]
            )
        gtile = gpool.tile([BPC, TC * S], f32, tag="gchunk")
        for j in range(NPAIR):
            scr = zscr.tile([128, C], f32, tag="zscratch")
            nc.scalar.activation(
                out=scr[:, :],
                in_=ytile[:, j, :],
                func=Act.Copy,
                bias=EPS,
                accum_out=zbig[:, ch * NPAIR + j : ch * NPAIR + j + 1],
            )
            g2 = g2pool.tile([128, SG], f32, tag="g2")
            # Absorb the gather's sync waits (DMA RAW on ytile, swap-DMA WAR on
            # g2) into a cheap same-engine op: the IndirectCopy ISA struct has
            # too few sync-wait slots for Tile's generated waits.
            nc.gpsimd.tensor_copy(out=g2[0:16, 0:1], in_=ytile[0:16, j, 0:1])
            if variant == "nogather":
                nc.gpsimd.tensor_copy(out=g2[:, :], in_=ytile[:, j, 0:SG])
            else:
                nc.gpsimd.indirect_copy(
                    g2[:, :],
                    ytile[:, j, :],
                    idx_sb[:, j * IDXW : (j + 1) * IDXW],
                    True,
                )
            nc.sync.dma_start(out=gtile[2 * j : 2 * j + 1, :], in_=g2[0:64, 0:S])
            nc.sync.dma_start(out=gtile[2 * j + 1 : 2 * j + 2, :], in_=g2[64:128, 0:S])
        gtiles.append(gtile)

    # --- the DP chain -----------------------------------------------------
    qa = singles.tile([BPC, SPAD], f32)
    qb = singles.tile([BPC, SPAD], f32)
    xt = singles.tile([BPC, S], f32)
    yt = singles.tile([BPC, S], f32)
    ut = singles.tile([BPC, S], f32)
    nrenorm = (T - 2) // RENORM  # renorms measured at t%8==7, t<511
    zstash = singles.tile([BPC, nrenorm], f32)

    nc.vector.memset(qa[:, :], 0.0)
    nc.vector.memset(qb[:, 0:2], 0.0)
    # q0 = g_0 at s in {0,1}
    nc.vector.tensor_copy(out=qa[:, 2:4], in_=gtiles[0][:, 0:2])

    rz_tiles = {}
    cur, nxt = qa, qb
    nsteps = 1 if variant == "nochain" else T
    for t in range(1, nsteps):
        ch, toff = divmod(t, TC)
        g_slice = gtiles[ch][:, toff * S : (toff + 1) * S]
        nc.vector.tensor_tensor(out=yt[:, :], in0=m_sb[:, :], in1=cur[:, 0:S], op=Alu.mult)
        nc.vector.tensor_tensor(
            out=xt[:, :], in0=cur[:, 2:SPAD], in1=cur[:, 1 : S + 1], op=Alu.add
        )
        nc.vector.tensor_tensor(out=ut[:, :], in0=xt[:, :], in1=yt[:, :], op=Alu.add)
        k, phase = divmod(t, RENORM)
        if variant == "chain_tt":
            nc.vector.tensor_tensor(
                out=nxt[:, 2:SPAD], in0=ut[:, :], in1=g_slice, op=Alu.mult
            )
            if phase == RENORM - 1:
                nc.vector.tensor_scalar(
                    out=nxt[:, 2:SPAD],
                    in0=nxt[:, 2:SPAD],
                    scalar1=1e-10,
                    scalar2=1e10,
                    op0=Alu.max,
                    op1=Alu.min,
                )
        elif phase == RENORM - 1 and k < nrenorm:
            # note: tensor_tensor_reduce would fuse these two, but its ISA
            # encoding fails at runtime on this stack — keep them separate
            nc.vector.tensor_tensor(
                out=nxt[:, 2:SPAD], in0=ut[:, :], in1=g_slice, op=Alu.mult
            )
            nc.vector.reduce_max(
                out=zstash[:, k : k + 1],
                in_=nxt[:, 2:SPAD],
                axis=mybir.AxisListType.X,
            )
            rz = small.tile([BPC, 1], f32, tag="rz")
            nc.vector.reciprocal(out=rz[:, :], in_=zstash[:, k : k + 1])
            rz_tiles[k] = rz
        elif phase == 0 and (t // RENORM - 1) in rz_tiles:
            rz = rz_tiles[t // RENORM - 1]
            nc.vector.scalar_tensor_tensor(
                out=nxt[:, 2:SPAD],
                in0=ut[:, :],
                scalar=rz[:, :],
                in1=g_slice,
                op0=Alu.mult,
                op1=Alu.mult,
            )
        else:
            nc.vector.tensor_tensor(
                out=nxt[:, 2:SPAD], in0=ut[:, :], in1=g_slice, op=Alu.mult
            )
        cur, nxt = nxt, cur

    # --- epilogue: loss = W - r - ln(q[S-1] + q[S-2]) ---------------------
    if variant in ("nochain", "chain_tt"):
        # dummy values so the Ln/reduce epilogue stays finite
        nc.vector.memset(zstash[:, :], 1.0)
        if variant == "nochain":
            nc.vector.memset(cur[:, SPAD - 2 : SPAD], 1.0)
    qsum = finp.tile([BPC, 1], f32, tag="fin")
    nc.vector.tensor_tensor(
        out=qsum[:, :], in0=cur[:, SPAD - 1 : SPAD], in1=cur[:, SPAD - 2 : SPAD - 1], op=Alu.add
    )
    lnq = finp.tile([BPC, 1], f32, tag="fin")
    nc.scalar.activation(out=lnq[:, :], in_=qsum[:, :], func=Act.Ln)
    lnz = finp.tile([BPC, nrenorm], f32, tag="lnz")
    nc.scalar.activation(out=lnz[:, :], in_=zstash[:, :], func=Act.Ln)
    r = finp.tile([BPC, 1], f32, tag="fin")
    nc.vector.reduce_sum(out=r[:, :], in_=lnz[:, :], axis=mybir.AxisListType.X)

    lnZ = singles.tile([128, NCHUNK * NPAIR], f32)
    nc.scalar.activation(out=lnZ[:, :], in_=zbig[:, :], func=Act.Ln)
    wsum = singles.tile([128, NPAIR], f32)
    lnZ_v = lnZ[:, :].rearrange("p (c q) -> p q c", c=NCHUNK)
    nc.vector.reduce_sum(out=wsum[:, :], in_=lnZ_v, axis=mybir.AxisListType.X)
    psw = psump.tile([NPAIR, 2], f32)
    nc.tensor.matmul(psw[:, :], lhsT=wsum[:, :], rhs=halfsel[:, :], start=True, stop=True)
    wpsb = finp.tile([NPAIR, 2], f32, tag="wpsb")
    nc.vector.tensor_copy(out=wpsb[:, :], in_=psw[:, :])
    wb = finp.tile([BPC, 1], f32, tag="fin")
    nc.sync.dma_start(out=wb[:, :], in_=wpsb[:, :])

    t1 = finp.tile([BPC, 1], f32, tag="fin")
    nc.vector.tensor_tensor(out=t1[:, :], in0=wb[:, :], in1=r[:, :], op=Alu.subtract)
    lt = finp.tile([BPC, 1], f32, tag="fin")
    nc.vector.tensor_tensor(out=lt[:, :], in0=t1[:, :], in1=lnq[:, :], op=Alu.subtract)
    nc.sync.dma_start(out=losst, in_=lt[:, :])


@functools.lru_cache(maxsize=4)
def _build(variant="full"):
    from contextlib import ExitStack

    import concourse.bacc as bacc
    import concourse.mybir as mybir
    import concourse.tile as tile

    nc = bacc.Bacc(trn_type="TRN2", target_bir_lowering=False)
    ypred = nc.dram_tensor("y_pred", [BPC, T, C], mybir.dt.float32, kind="ExternalInput")
    idxt = nc.dram_tensor(
        "idx", [128, NPAIR * IDXW], mybir.dt.uint16, kind="ExternalInput"
    )
    maskt = nc.dram_tensor("mask", [BPC, S], mybir.dt.float32, kind="ExternalInput")
    losst = nc.dram_tensor("loss", [BPC, 1], mybir.dt.float32, kind="ExternalOutput")
    with tile.TileContext(nc) as tc:
        with ExitStack() as ctx:
            _emit_kernel(
                ctx, tc, ypred[:, :, :], idxt[:, :], maskt[:, :], losst[:, :], variant
            )
    nc.compile()
    return nc


def _host_prep(y_true):
    """Per-core wrapped gather indices and skip-transition masks."""
    y_true = np.asarray(y_true).astype(np.int64)
    ext = np.full((B, S), BLANK, dtype=np.int64)
    ext[:, 1::2] = y_true
    mask = np.zeros((B, S), dtype=np.float32)
    mask[:, 1] = 1.0
    lab = y_true
    neq = (lab[:, 1:] != lab[:, :-1]).astype(np.float32)
    mask[:, 3::2] = neq

    idx_all = []
    for k in range(NCORES):
        idx = np.zeros((128, NPAIR * IDXW), dtype=np.uint16)
        base = k * BPC
        p = np.arange(128)
        for j in range(NPAIR):
            b = base + 2 * j + (p >= 64).astype(np.int64)
            for f in range(IDXW):
                pos = f * 16 + (p % 16)
                valid = pos < S
                idx[p[valid], j * IDXW + f] = ext[b[valid], pos[valid]]
        idx_all.append(idx)
    return idx_all, mask


def kernel(y_true, y_pred):
    from concourse.bass_utils import run_bass_kernel_spmd

    y_pred = np.ascontiguousarray(np.asarray(y_pred), dtype=np.float32)
    idx_all, mask = _host_prep(y_true)

    nc = _build(os.environ.get("CTC_VARIANT", "full"))
    in_maps = []
    for k in range(NCORES):
        b0 = k * BPC
        in_maps.append(
            {
                "y_pred": np.ascontiguousarray(y_pred[b0 : b0 + BPC]),
                "idx": idx_all[k],
                "mask": np.ascontiguousarray(mask[b0 : b0 + BPC]),
            }
        )
    res = run_bass_kernel_spmd(
        nc,
        in_maps,
        core_ids=list(range(NCORES)),
        trace=bool(int(os.environ.get("CTC_TRACE", "0"))),
    )
    out = np.concatenate([r["loss"] for r in res.results], axis=0)
    if res.exec_time_ns is not None:
        print(f"HW exec time: {res.exec_time_ns} ns", file=sys.stderr)
    return out.astype(np.float32)
